# revision 18
# baseline (speedup 1.0000x reference)
"""BertSelfAttention fused kernel for Trainium2, 8 NeuronCores.

Sharding: tensor-parallel over heads. 16 heads / 8 cores = 2 heads per core.
Core c owns heads 2c, 2c+1 == output feature columns [128c, 128c+128).
Every core reads the full hidden_states (pre-transposed on host to [D, B*S])
plus its 128-column slice of Wq/Wk/Wv (pre-transposed to [D, 128]); it writes
its [B*S, 128] slab of the output. No cross-core communication.

Per-core device program (B=4 batches, S=2048, D=1024, HD=64):
  stage 0: load weights/biases/mask constants; f = exp(mask) per key.
  per batch b:
    stage 1 (projection): QT,KT [128, 2048] (partition = head-elem dim,
      2 heads stacked), V [128 tok, 16 kblk, 130] where cols 0:64 head A,
      64 = denom col, 65:129 head B, 129 = denom col; V rows scaled by
      f=exp(mask) (folds the additive attention mask into the softmax
      weights exactly) and the denom cols set to f, so the softmax
      denominator comes out of the PV matmul for free.
    stage 2 (attention), per 512-query group, software-pipelined:
      for each 128-key block: S^T = K^T.T @ Q^T  (PSUM, keys on
        partitions; the two heads run as concurrent PE row-tiles)
        E = exp(S^T / 8)  (ACT, PSUM->SBUF, one [128,1024] pass, 2 heads)
        ctx~ += [V|f].T @ E  (PSUM accumulate, [65, 512] per head;
                              row 64 = softmax denominator)
      epilogue: PE-transpose ctx~ 128-query chunks, out = num/den (DVE),
        DMA [128, 128] (2 heads) to the output slab.

Matmul operands are fp16 (1 PE cycle/column vs 2 for fp32r tf32 and 4
for fp32; fp16's 10 mantissa bits keep the output within ~5e-4 of the
fp32 reference; all tensors here fit fp16 range). PSUM accumulation is
always fp32.
"""

import sys

sys.path.insert(0, "/opt/trn_rl_repo")

from contextlib import ExitStack

import numpy as np

import concourse.bass as bass
import concourse.mybir as mybir
import concourse.tile as tile
from concourse import bacc
from concourse.bass import ds
from concourse.masks import make_identity

B, S, D = 4, 2048, 1024
H, HD = 16, 64
NCORES = 8
CW = 128  # output columns per core (2 heads * 64)
P = 128

FP32 = mybir.dt.float32
FP32R = mybir.dt.float32r
BF16 = mybir.dt.bfloat16
FP16 = mybir.dt.float16

# matmul-operand dtype: "fp32" (exact, 4 cyc/col), "fp32r" (tf32, 2 cyc/col),
# "bf16"/"fp16" (1 cyc/col; fp16 carries 10 mantissa bits vs bf16's 8 and all
# tensors here fit fp16 range), "mixed" (projections fp32r, attention fp16)
MM_DTYPE = "fp16"

# key-blocks (of 16 per query group) whose exp runs on the vector engine as a
# one-instruction fp16 Schraudolph (bits = s*A + B, ~2% rel err on those
# blocks; full-pipeline numpy sim: 1.08e-2 final max rel err vs 2e-2 budget).
# Batches 0-2 overlap the next batch's projections (PE-paced, ACT has slack)
# so they offload little; batch 3 has no projections left (ACT-paced) and
# offloads more to pull ACT off the critical path.
def _dve_exp_kbs(b, b_sz):
    if b == b_sz - 1:
        return frozenset({1, 4, 6, 9, 11, 14})
    return frozenset({5, 11})
SCHRAUD_A = 1024.0 / float(np.log(2.0)) / 8.0   # fold the 1/8 score scale
SCHRAUD_B = 15360.0 - 44.0


def _mm_dts(mm_dtype):
    """-> (projection operand dtype, attention operand dtype)"""
    if mm_dtype == "mixed":
        return FP32R, FP16
    dt = {"fp32": FP32, "fp32r": FP32R, "bf16": BF16, "fp16": FP16}[mm_dtype]
    return dt, dt


def emit_kernel(ctx: ExitStack, tc: tile.TileContext, aps: dict, b_sz: int,
                s_sz: int, mm_dtype: str):
    nc = tc.nc
    n_tok = b_sz * s_sz
    TB = min(512, s_sz)           # projection token-block / query-group size
    n_tb = s_sz // TB             # token blocks per batch
    n_kb = s_sz // P              # key blocks per batch
    n_qg = s_sz // TB             # query groups per batch
    DCH = D // P                  # contraction chunks (8)
    n_bk = b_sz * n_kb            # total key blocks

    PJ, AT = _mm_dts(mm_dtype)
    hid_t, wqt, wkt, wvt, bq, bk, bv, mask, out = (
        aps["hidden_t"], aps["wqt"], aps["wkt"], aps["wvt"], aps["bq"],
        aps["bk"], aps["bv"], aps["mask"], aps["out"])

    const = ctx.enter_context(tc.tile_pool(name="const", bufs=1))
    hidp = ctx.enter_context(tc.tile_pool(name="hidp", bufs=4))
    qkv = ctx.enter_context(tc.tile_pool(name="qkv", bufs=6))
    epool = ctx.enter_context(tc.tile_pool(name="epool", bufs=6))
    csb = ctx.enter_context(tc.tile_pool(name="csb", bufs=3))
    psA = ctx.enter_context(tc.tile_pool(name="psA", bufs=2, space="PSUM"))
    psC = ctx.enter_context(tc.tile_pool(name="psC", bufs=1, space="PSUM"))
    psP = ctx.enter_context(tc.tile_pool(name="psP", bufs=2, space="PSUM"))

    # ---- stage 0: constants ----
    # Emission order tuned for startup: the first projection chain needs
    # wq + hid(0), so those DMAs go first on the queue; the identity (which
    # gates PE's first instruction, the mask transpose) is built on gpsimd
    # before the bv-broadcast DMA is queued there.
    wq_sb = const.tile([P, DCH, CW], PJ)
    nc.sync.dma_start(wq_sb, wqt.rearrange("(c p) m -> p c m", p=P))
    bq_sb = const.tile([P, 1], FP32)
    nc.sync.dma_start(bq_sb, bq.rearrange("(p o) -> p o", o=1))
    mask_bo = const.tile([n_bk, P], FP32)
    nc.sync.dma_start(mask_bo, mask.rearrange("b (o p) -> (b o) p", p=P))

    ident = const.tile([P, P], FP32)
    make_identity(nc, ident)

    # mask: [b_sz, s_sz] -> keys-on-partitions [128, b_sz * n_kb]; it feeds
    # the exp as a per-partition (per-key) ACT bias: exp(s/8 + mask), so no
    # f = exp(mask) scaling of V is needed anywhere
    mask_ps = psP.tile([P, n_bk], FP32, tag="proj", name="mask_ps")
    nc.tensor.matmul(mask_ps, mask_bo, ident[:n_bk, :n_bk], is_transpose=True)
    mask_sb = const.tile([P, n_bk], FP32)
    nc.vector.tensor_copy(mask_sb, mask_ps)
    # per-key Schraudolph bias: B + mask * (1024/ln2)
    sch_b = const.tile([P, n_bk], FP32)
    nc.vector.tensor_scalar(sch_b, mask_sb, 8.0 * SCHRAUD_A, SCHRAUD_B,
                            mybir.AluOpType.mult, mybir.AluOpType.add)

    wk_sb = const.tile([P, DCH, CW], PJ)
    nc.sync.dma_start(wk_sb, wkt.rearrange("(c p) m -> p c m", p=P))
    wv_sb = const.tile([P, DCH, CW], PJ)
    nc.sync.dma_start(wv_sb, wvt.rearrange("(c p) m -> p c m", p=P))
    bk_sb = const.tile([P, 1], FP32)
    nc.sync.dma_start(bk_sb, bk.rearrange("(p o) -> p o", o=1))

    for b in range(b_sz):
        # ---- stage 1: projections for batch b ----
        qt_b = qkv.tile([P, s_sz], AT, tag="qt", name="qt_b")
        kt_b = qkv.tile([P, s_sz], AT, tag="kt", name="kt_b")
        v_b = qkv.tile([P, n_kb, 130], AT, tag="v", name="v_b")
        # denominator columns (64 and 129 of each key-block) are constant 1.0
        nc.vector.memset(v_b[:, :, ds(HD, 1)], 1.0)
        nc.vector.memset(v_b[:, :, ds(129, 1)], 1.0)

        for tb in range(n_tb):
            tok0 = b * s_sz + tb * TB
            hid_tile = hidp.tile([P, DCH, TB], PJ, tag="hid", name="hid_tile")
            # two half-depth DMAs so the first accumulation chunks can start
            # before the whole [1024, TB] slab has landed
            # hid tiles ride the (otherwise idle) gpsimd DMA queue so they
            # never sit behind weight/const/output DMAs on the sync queue
            hid_src = hid_t.rearrange("(c p) n -> p c n", p=P)[:, :, ds(tok0, TB)]
            nc.gpsimd.dma_start(hid_tile[:, 0:DCH // 2], hid_src[:, 0:DCH // 2])
            nc.gpsimd.dma_start(hid_tile[:, DCH // 2:DCH],
                                hid_src[:, DCH // 2:DCH])

            pq = psP.tile([P, TB], FP32, tag="proj", name="pq")
            for c in range(DCH):
                nc.tensor.matmul(pq, wq_sb[:, c, :],
                                 hid_tile[:, c, :],
                                 start=(c == 0), stop=(c == DCH - 1))
            nc.vector.tensor_scalar_add(qt_b[:, ds(tb * TB, TB)], pq, bq_sb)

            pk = psP.tile([P, TB], FP32, tag="proj", name="pk")
            for c in range(DCH):
                nc.tensor.matmul(pk, wk_sb[:, c, :],
                                 hid_tile[:, c, :],
                                 start=(c == 0), stop=(c == DCH - 1))
            nc.vector.tensor_scalar_add(kt_b[:, ds(tb * TB, TB)], pk, bk_sb)

            for s4 in range(TB // P):
                kbg = tb * (TB // P) + s4  # key block index within batch
                pv = psP.tile([P, CW], FP32, tag="proj", name="pv")
                for c in range(DCH):
                    nc.tensor.matmul(
                        pv, hid_tile[:, c, ds(s4 * P, P)],
                        wv_sb[:, c, :],
                        start=(c == 0), stop=(c == DCH - 1))
                # V bias is folded out on the host (num + bv*den), and the
                # mask folds into the exp bias, so V is just a cast
                nc.vector.tensor_copy(v_b[:, kbg, 0:HD], pv[:, 0:HD])
                nc.vector.tensor_copy(v_b[:, kbg, 65:129], pv[:, HD:CW])

        # ---- stage 2: attention for batch b ----
        for qg in range(n_qg):
            q0 = qg * TB
            ctx_ps = psC.tile([P, 2 * TB], FP32, tag="ctx", name="ctx_ps")

            def emit_scores(kb):
                st = psA.tile([P, 2 * TB], FP32, tag="st", name="st")
                nc.tensor.matmul(st[:, 0:TB],
                                 kt_b[0:HD, ds(kb * P, P)],
                                 qt_b[0:HD, ds(q0, TB)],
                                 start=True, stop=True)
                nc.tensor.matmul(st[:, ds(TB, TB)],
                                 kt_b[HD:P, ds(kb * P, P)],
                                 qt_b[HD:P, ds(q0, TB)],
                                 start=True, stop=True)
                return st

            # software-pipelined with a single sem gate per kb on the PE
            # queue: exp(kb)-done unblocks BOTH pv(kb) and scores(kb+2) (the
            # latter reuses the st buffer exp(kb) just drained), so the PE
            # pays one resume-latency tail per kb instead of two
            sts = [emit_scores(0), emit_scores(1)]
            for kb in range(n_kb):
                e_t = epool.tile([P, 2 * TB], AT, tag="e", name="e_t")
                mcol = ds(b * n_kb + kb, 1)
                if kb in _dve_exp_kbs(b, b_sz) and AT == FP16:
                    # fp16 Schraudolph on the vector engine: the fp16 bit
                    # pattern of ~exp(s/8 + mask) is the per-partition affine
                    # int16(s*A + (B + mask*8A)); offloads 3/16 of the exp
                    # stream from the saturated ACT
                    nc.vector.tensor_scalar(
                        e_t.bitcast(mybir.dt.int16), sts[kb],
                        SCHRAUD_A, sch_b[:, mcol],
                        mybir.AluOpType.mult, mybir.AluOpType.add)
                else:
                    # mask rides the ACT affine: exp(s/8 + mask)
                    nc.scalar.activation(e_t, sts[kb],
                                         mybir.ActivationFunctionType.Exp,
                                         bias=mask_sb[:, mcol],
                                         scale=1.0 / 8.0)
                nc.tensor.matmul(ctx_ps[0:65, 0:TB],
                                 v_b[:, kb, 0:65],
                                 e_t[:, 0:TB],
                                 start=(kb == 0), stop=(kb == n_kb - 1))
                nc.tensor.matmul(ctx_ps[0:65, ds(TB, TB)],
                                 v_b[:, kb, ds(65, 65)],
                                 e_t[:, ds(TB, TB)],
                                 start=(kb == 0), stop=(kb == n_kb - 1))
                if kb + 2 < n_kb:
                    sts.append(emit_scores(kb + 2))

            # evacuate the [65, 2*TB] numerator/denominator block to SBUF and
            # DMA it out feature-major; the host does num/den + transpose
            ctx_sb = csb.tile([65, 2 * TB], FP32, tag="csb", name="ctx_sb")
            nc.vector.tensor_copy(ctx_sb, ctx_ps[0:65, :])
            tok0 = b * s_sz + q0
            for j in range(2):
                nc.sync.dma_start(out[j, :, ds(tok0, TB)],
                                  ctx_sb[:, ds(j * TB, TB)])


def build_program(b_sz=B, s_sz=S, mm_dtype=None):
    if mm_dtype is None:
        mm_dtype = MM_DTYPE
    PJ, _ = _mm_dts(mm_dtype)
    nc = bacc.Bacc("TRN2", target_bir_lowering=False, debug=False)
    n_tok = b_sz * s_sz
    aps = {
        "hidden_t": nc.dram_tensor("hidden_t", [D, n_tok], PJ,
                                   kind="ExternalInput").ap(),
        "wqt": nc.dram_tensor("wqt", [D, CW], PJ, kind="ExternalInput").ap(),
        "wkt": nc.dram_tensor("wkt", [D, CW], PJ, kind="ExternalInput").ap(),
        "wvt": nc.dram_tensor("wvt", [D, CW], PJ, kind="ExternalInput").ap(),
        "bq": nc.dram_tensor("bq", [CW], FP32, kind="ExternalInput").ap(),
        "bk": nc.dram_tensor("bk", [CW], FP32, kind="ExternalInput").ap(),
        "bv": nc.dram_tensor("bv", [CW], FP32, kind="ExternalInput").ap(),
        "mask": nc.dram_tensor("mask", [b_sz, s_sz], FP32,
                               kind="ExternalInput").ap(),
        # feature-major numerator+denominator: out[j, 0:64, tok] = head-j
        # context numerator, out[j, 64, tok] = softmax denominator
        "out": nc.dram_tensor("out", [2, 65, n_tok], FP32,
                              kind="ExternalOutput").ap(),
    }
    with tile.TileContext(nc) as tc:
        with ExitStack() as ctx:
            emit_kernel(ctx, tc, aps, b_sz, s_sz, mm_dtype)
    nc.compile()
    return nc


def make_in_maps(hidden_states, attention_mask, Wq, bq, Wk, bk, Wv, bv,
                 b_sz=B, s_sz=S):
    hidden_states = np.asarray(hidden_states, dtype=np.float32)
    x = hidden_states.reshape(b_sz * s_sz, D)
    hid_t = np.ascontiguousarray(x.T)
    mask = np.ascontiguousarray(
        np.broadcast_to(
            np.asarray(attention_mask, dtype=np.float32).reshape(b_sz, 1, 1,
                                                                 s_sz),
            (b_sz, 1, 1, s_sz)).reshape(b_sz, s_sz))
    Wq, Wk, Wv = (np.asarray(w, dtype=np.float32) for w in (Wq, Wk, Wv))
    bq, bk, bv = (np.asarray(v, dtype=np.float32) for v in (bq, bk, bv))
    # hidden/weights are pre-cast on the host to the projection matmul dtype
    # (same round-to-nearest the device casting DMA would apply)
    pj_np = mybir.dt.np(_mm_dts(MM_DTYPE)[0])
    hid_t = hid_t.astype(pj_np)
    in_maps = []
    for c in range(NCORES):
        rows = slice(c * CW, (c + 1) * CW)
        in_maps.append({
            "hidden_t": hid_t,
            "wqt": np.ascontiguousarray(Wq[rows, :].T).astype(pj_np),
            "wkt": np.ascontiguousarray(Wk[rows, :].T).astype(pj_np),
            "wvt": np.ascontiguousarray(Wv[rows, :].T).astype(pj_np),
            "bq": np.ascontiguousarray(bq[rows]),
            "bk": np.ascontiguousarray(bk[rows]),
            "bv": np.ascontiguousarray(bv[rows]),
            "mask": mask,
        })
    return in_maps


_NC_CACHE = {}


def _get_program():
    key = (B, S, MM_DTYPE)
    if key not in _NC_CACHE:
        _NC_CACHE[key] = build_program(B, S, MM_DTYPE)
    return _NC_CACHE[key]


def assemble_out(results, bv, b_sz=B, s_sz=S):
    """results[c]["out"] is [2, 65, n_tok] fp32 (num rows 0:64, den row 64,
    feature-major). Host finishes the softmax division, adds the V bias
    (sum_k (V+bv)*E = num + bv*den, so out = num/den + bv), transposes."""
    bv = np.asarray(bv, dtype=np.float32)
    out = np.empty((b_sz * s_sz, D), dtype=np.float32)
    for c in range(NCORES):
        o = results[c]["out"]
        for j in range(2):
            num = o[j, 0:HD, :]                      # [64, n_tok]
            den = o[j, HD, :]                        # [n_tok]
            col0 = c * CW + j * HD
            out[:, col0:col0 + HD] = (num / den).T + bv[col0:col0 + HD]
    return out.reshape(b_sz, s_sz, D)


def kernel(hidden_states, attention_mask, Wq, bq, Wk, bk, Wv, bv):
    from concourse.bass_utils import run_bass_kernel_spmd

    nc = _get_program()
    in_maps = make_in_maps(hidden_states, attention_mask, Wq, bq, Wk, bk, Wv,
                           bv)
    res = run_bass_kernel_spmd(nc, in_maps, list(range(NCORES)))
    return assemble_out(res.results, bv)



# revision 20
# speedup vs baseline: 1.0034x; 1.0034x over previous
"""BertSelfAttention fused kernel for Trainium2, 8 NeuronCores.

Sharding: tensor-parallel over heads. 16 heads / 8 cores = 2 heads per core.
Core c owns heads 2c, 2c+1 == output feature columns [128c, 128c+128).
Every core reads the full hidden_states (pre-transposed on host to [D, B*S])
plus its 128-column slice of Wq/Wk/Wv (pre-transposed to [D, 128]); it writes
a feature-major [2, 65, B*S] numerator/denominator slab, and the host
finishes the softmax division + V-bias + transpose (out = num/den + bv,
since sum_k (V+bv)*E = num + bv*den). No cross-core communication.

Per-core device program (B=4 batches, S=2048, D=1024, HD=64):
  stage 0: load weights/biases/mask constants; mask goes keys-on-partitions
    so it can ride the exp as a per-partition ACT bias: E = exp(s/8 + mask).
  per batch b (batch b+1's projections overlap batch b's attention; hid
  tiles prefetch on the gpsimd DMA queue):
    stage 1 (projection): QT,KT [128, 2048] (partition = head-elem dim,
      2 heads stacked), V [128 tok, 16 kblk, 130] where cols 0:64 head A,
      64 = denom col (constant 1.0), 65:129 head B, 129 = denom col, so the
      softmax denominator comes out of the PV matmul for free.
    stage 2 (attention), per 512-query group, software-pipelined at
    key-block granularity (PSUM: scores 2x2 banks, ctx 2, proj 2):
      for each 128-key block: S^T = K^T.T @ Q^T  (PSUM, keys on
        partitions; the two heads run as concurrent PE row-tiles)
        E = exp(S^T / 8 + mask)  (ACT, PSUM->SBUF fp16, one [128,1024]
        pass for both heads; 3 of 16 key-blocks instead compute E on the
        vector engine as a one-instruction fp16 Schraudolph to keep the
        saturated ACT off the critical path)
        ctx~ += [V|1].T @ E  (PSUM accumulate, [65, 512] per head;
                              row 64 = softmax denominator)
      epilogue: one DVE copy [65, 1024] PSUM->SBUF, two DMAs to the
        feature-major output slab.

Matmul operands are fp16 (1 PE cycle/column vs 2 for fp32r tf32 and 4
for fp32; fp16's 10 mantissa bits keep the output within ~5e-4 of the
fp32 reference before the Schraudolph blocks, ~8e-3 with them; all
tensors here fit fp16 range). PSUM accumulation is always fp32.
"""

import sys

sys.path.insert(0, "/opt/trn_rl_repo")

from contextlib import ExitStack

import numpy as np

import concourse.bass as bass
import concourse.mybir as mybir
import concourse.tile as tile
from concourse import bacc
from concourse.bass import ds
from concourse.masks import make_identity

B, S, D = 4, 2048, 1024
H, HD = 16, 64
NCORES = 8
CW = 128  # output columns per core (2 heads * 64)
P = 128

FP32 = mybir.dt.float32
FP32R = mybir.dt.float32r
BF16 = mybir.dt.bfloat16
FP16 = mybir.dt.float16

# matmul-operand dtype: "fp32" (exact, 4 cyc/col), "fp32r" (tf32, 2 cyc/col),
# "bf16"/"fp16" (1 cyc/col; fp16 carries 10 mantissa bits vs bf16's 8 and all
# tensors here fit fp16 range), "mixed" (projections fp32r, attention fp16)
MM_DTYPE = "fp16"

# key-blocks (of 16 per query group) whose exp runs on the vector engine as a
# one-instruction fp16 Schraudolph (bits = s*A + B, ~2% rel err on those
# blocks; full-pipeline numpy sim: 8.1e-3 final max rel err vs 2e-2 budget)
def _dve_exp_kbs(b, b_sz):
    return frozenset({2, 7, 12})
SCHRAUD_A = 1024.0 / float(np.log(2.0)) / 8.0   # fold the 1/8 score scale
SCHRAUD_B = 15360.0 - 44.0


def _mm_dts(mm_dtype):
    """-> (projection operand dtype, attention operand dtype)"""
    if mm_dtype == "mixed":
        return FP32R, FP16
    dt = {"fp32": FP32, "fp32r": FP32R, "bf16": BF16, "fp16": FP16}[mm_dtype]
    return dt, dt


def emit_kernel(ctx: ExitStack, tc: tile.TileContext, aps: dict, b_sz: int,
                s_sz: int, mm_dtype: str):
    nc = tc.nc
    n_tok = b_sz * s_sz
    TB = min(512, s_sz)           # projection token-block / query-group size
    n_tb = s_sz // TB             # token blocks per batch
    n_kb = s_sz // P              # key blocks per batch
    n_qg = s_sz // TB             # query groups per batch
    DCH = D // P                  # contraction chunks (8)
    n_bk = b_sz * n_kb            # total key blocks

    PJ, AT = _mm_dts(mm_dtype)
    hid_t, wqt, wkt, wvt, bq, bk, bv, mask, out = (
        aps["hidden_t"], aps["wqt"], aps["wkt"], aps["wvt"], aps["bq"],
        aps["bk"], aps["bv"], aps["mask"], aps["out"])

    const = ctx.enter_context(tc.tile_pool(name="const", bufs=1))
    hidp = ctx.enter_context(tc.tile_pool(name="hidp", bufs=4))
    qkv = ctx.enter_context(tc.tile_pool(name="qkv", bufs=6))
    epool = ctx.enter_context(tc.tile_pool(name="epool", bufs=6))
    csb = ctx.enter_context(tc.tile_pool(name="csb", bufs=3))
    psA = ctx.enter_context(tc.tile_pool(name="psA", bufs=2, space="PSUM"))
    psC = ctx.enter_context(tc.tile_pool(name="psC", bufs=1, space="PSUM"))
    psP = ctx.enter_context(tc.tile_pool(name="psP", bufs=2, space="PSUM"))

    # ---- stage 0: constants ----
    # Emission order tuned for startup: the first projection chain needs
    # wq + hid(0), so those DMAs go first on the queue; the identity (which
    # gates PE's first instruction, the mask transpose) is built on gpsimd
    # before the bv-broadcast DMA is queued there.
    wq_sb = const.tile([P, DCH, CW], PJ)
    nc.sync.dma_start(wq_sb, wqt.rearrange("(c p) m -> p c m", p=P))
    bq_sb = const.tile([P, 1], FP32)
    nc.sync.dma_start(bq_sb, bq.rearrange("(p o) -> p o", o=1))
    mask_bo = const.tile([n_bk, P], FP32)
    nc.sync.dma_start(mask_bo, mask.rearrange("b (o p) -> (b o) p", p=P))

    ident = const.tile([P, P], FP32)
    make_identity(nc, ident)

    # mask: [b_sz, s_sz] -> keys-on-partitions [128, b_sz * n_kb]; it feeds
    # the exp as a per-partition (per-key) ACT bias: exp(s/8 + mask), so no
    # f = exp(mask) scaling of V is needed anywhere
    mask_ps = psP.tile([P, n_bk], FP32, tag="proj", name="mask_ps")
    nc.tensor.matmul(mask_ps, mask_bo, ident[:n_bk, :n_bk], is_transpose=True)
    mask_sb = const.tile([P, n_bk], FP32)
    nc.vector.tensor_copy(mask_sb, mask_ps)
    # per-key Schraudolph bias: B + mask * (1024/ln2)
    sch_b = const.tile([P, n_bk], FP32)
    nc.vector.tensor_scalar(sch_b, mask_sb, 8.0 * SCHRAUD_A, SCHRAUD_B,
                            mybir.AluOpType.mult, mybir.AluOpType.add)

    wk_sb = const.tile([P, DCH, CW], PJ)
    nc.sync.dma_start(wk_sb, wkt.rearrange("(c p) m -> p c m", p=P))
    wv_sb = const.tile([P, DCH, CW], PJ)
    nc.sync.dma_start(wv_sb, wvt.rearrange("(c p) m -> p c m", p=P))
    bk_sb = const.tile([P, 1], FP32)
    nc.sync.dma_start(bk_sb, bk.rearrange("(p o) -> p o", o=1))

    for b in range(b_sz):
        # ---- stage 1: projections for batch b ----
        qt_b = qkv.tile([P, s_sz], AT, tag="qt", name="qt_b")
        kt_b = qkv.tile([P, s_sz], AT, tag="kt", name="kt_b")
        v_b = qkv.tile([P, n_kb, 130], AT, tag="v", name="v_b")
        # denominator columns (64 and 129 of each key-block) are constant 1.0
        nc.vector.memset(v_b[:, :, ds(HD, 1)], 1.0)
        nc.vector.memset(v_b[:, :, ds(129, 1)], 1.0)

        for tb in range(n_tb):
            tok0 = b * s_sz + tb * TB
            hid_tile = hidp.tile([P, DCH, TB], PJ, tag="hid", name="hid_tile")
            # two half-depth DMAs so the first accumulation chunks can start
            # before the whole [1024, TB] slab has landed
            # hid tiles ride the (otherwise idle) gpsimd DMA queue so they
            # never sit behind weight/const/output DMAs on the sync queue
            hid_src = hid_t.rearrange("(c p) n -> p c n", p=P)[:, :, ds(tok0, TB)]
            nc.gpsimd.dma_start(hid_tile[:, 0:DCH // 2], hid_src[:, 0:DCH // 2])
            nc.gpsimd.dma_start(hid_tile[:, DCH // 2:DCH],
                                hid_src[:, DCH // 2:DCH])

            pq = psP.tile([P, TB], FP32, tag="proj", name="pq")
            for c in range(DCH):
                nc.tensor.matmul(pq, wq_sb[:, c, :],
                                 hid_tile[:, c, :],
                                 start=(c == 0), stop=(c == DCH - 1))
            nc.vector.tensor_scalar_add(qt_b[:, ds(tb * TB, TB)], pq, bq_sb)

            pk = psP.tile([P, TB], FP32, tag="proj", name="pk")
            for c in range(DCH):
                nc.tensor.matmul(pk, wk_sb[:, c, :],
                                 hid_tile[:, c, :],
                                 start=(c == 0), stop=(c == DCH - 1))
            nc.vector.tensor_scalar_add(kt_b[:, ds(tb * TB, TB)], pk, bk_sb)

            for s4 in range(TB // P):
                kbg = tb * (TB // P) + s4  # key block index within batch
                pv = psP.tile([P, CW], FP32, tag="proj", name="pv")
                for c in range(DCH):
                    nc.tensor.matmul(
                        pv, hid_tile[:, c, ds(s4 * P, P)],
                        wv_sb[:, c, :],
                        start=(c == 0), stop=(c == DCH - 1))
                # V bias is folded out on the host (num + bv*den), and the
                # mask folds into the exp bias, so V is just a cast
                nc.vector.tensor_copy(v_b[:, kbg, 0:HD], pv[:, 0:HD])
                nc.vector.tensor_copy(v_b[:, kbg, 65:129], pv[:, HD:CW])

        # ---- stage 2: attention for batch b ----
        for qg in range(n_qg):
            q0 = qg * TB
            ctx_ps = psC.tile([P, 2 * TB], FP32, tag="ctx", name="ctx_ps")

            def emit_scores(kb):
                st = psA.tile([P, 2 * TB], FP32, tag="st", name="st")
                nc.tensor.matmul(st[:, 0:TB],
                                 kt_b[0:HD, ds(kb * P, P)],
                                 qt_b[0:HD, ds(q0, TB)],
                                 start=True, stop=True)
                nc.tensor.matmul(st[:, ds(TB, TB)],
                                 kt_b[HD:P, ds(kb * P, P)],
                                 qt_b[HD:P, ds(q0, TB)],
                                 start=True, stop=True)
                return st

            # software-pipelined with a single sem gate per kb on the PE
            # queue: exp(kb)-done unblocks BOTH pv(kb) and scores(kb+2) (the
            # latter reuses the st buffer exp(kb) just drained), so the PE
            # pays one resume-latency tail per kb instead of two
            sts = [emit_scores(0), emit_scores(1)]
            for kb in range(n_kb):
                e_t = epool.tile([P, 2 * TB], AT, tag="e", name="e_t")
                mcol = ds(b * n_kb + kb, 1)
                if kb in _dve_exp_kbs(b, b_sz) and AT == FP16:
                    # fp16 Schraudolph on the vector engine: the fp16 bit
                    # pattern of ~exp(s/8 + mask) is the per-partition affine
                    # int16(s*A + (B + mask*8A)); offloads 3/16 of the exp
                    # stream from the saturated ACT
                    nc.vector.tensor_scalar(
                        e_t.bitcast(mybir.dt.int16), sts[kb],
                        SCHRAUD_A, sch_b[:, mcol],
                        mybir.AluOpType.mult, mybir.AluOpType.add)
                else:
                    # mask rides the ACT affine: exp(s/8 + mask)
                    nc.scalar.activation(e_t, sts[kb],
                                         mybir.ActivationFunctionType.Exp,
                                         bias=mask_sb[:, mcol],
                                         scale=1.0 / 8.0)
                nc.tensor.matmul(ctx_ps[0:65, 0:TB],
                                 v_b[:, kb, 0:65],
                                 e_t[:, 0:TB],
                                 start=(kb == 0), stop=(kb == n_kb - 1))
                nc.tensor.matmul(ctx_ps[0:65, ds(TB, TB)],
                                 v_b[:, kb, ds(65, 65)],
                                 e_t[:, ds(TB, TB)],
                                 start=(kb == 0), stop=(kb == n_kb - 1))
                if kb + 2 < n_kb:
                    sts.append(emit_scores(kb + 2))

            # evacuate the [65, 2*TB] numerator/denominator block to SBUF and
            # DMA it out feature-major; the host does num/den + transpose
            ctx_sb = csb.tile([65, 2 * TB], FP32, tag="csb", name="ctx_sb")
            nc.vector.tensor_copy(ctx_sb, ctx_ps[0:65, :])
            tok0 = b * s_sz + q0
            for j in range(2):
                nc.sync.dma_start(out[j, :, ds(tok0, TB)],
                                  ctx_sb[:, ds(j * TB, TB)])


def build_program(b_sz=B, s_sz=S, mm_dtype=None):
    if mm_dtype is None:
        mm_dtype = MM_DTYPE
    PJ, _ = _mm_dts(mm_dtype)
    nc = bacc.Bacc("TRN2", target_bir_lowering=False, debug=False)
    n_tok = b_sz * s_sz
    aps = {
        "hidden_t": nc.dram_tensor("hidden_t", [D, n_tok], PJ,
                                   kind="ExternalInput").ap(),
        "wqt": nc.dram_tensor("wqt", [D, CW], PJ, kind="ExternalInput").ap(),
        "wkt": nc.dram_tensor("wkt", [D, CW], PJ, kind="ExternalInput").ap(),
        "wvt": nc.dram_tensor("wvt", [D, CW], PJ, kind="ExternalInput").ap(),
        "bq": nc.dram_tensor("bq", [CW], FP32, kind="ExternalInput").ap(),
        "bk": nc.dram_tensor("bk", [CW], FP32, kind="ExternalInput").ap(),
        "bv": nc.dram_tensor("bv", [CW], FP32, kind="ExternalInput").ap(),
        "mask": nc.dram_tensor("mask", [b_sz, s_sz], FP32,
                               kind="ExternalInput").ap(),
        # feature-major numerator+denominator: out[j, 0:64, tok] = head-j
        # context numerator, out[j, 64, tok] = softmax denominator
        "out": nc.dram_tensor("out", [2, 65, n_tok], FP32,
                              kind="ExternalOutput").ap(),
    }
    with tile.TileContext(nc) as tc:
        with ExitStack() as ctx:
            emit_kernel(ctx, tc, aps, b_sz, s_sz, mm_dtype)
    nc.compile()
    return nc


def make_in_maps(hidden_states, attention_mask, Wq, bq, Wk, bk, Wv, bv,
                 b_sz=B, s_sz=S):
    hidden_states = np.asarray(hidden_states, dtype=np.float32)
    x = hidden_states.reshape(b_sz * s_sz, D)
    hid_t = np.ascontiguousarray(x.T)
    mask = np.ascontiguousarray(
        np.broadcast_to(
            np.asarray(attention_mask, dtype=np.float32).reshape(b_sz, 1, 1,
                                                                 s_sz),
            (b_sz, 1, 1, s_sz)).reshape(b_sz, s_sz))
    Wq, Wk, Wv = (np.asarray(w, dtype=np.float32) for w in (Wq, Wk, Wv))
    bq, bk, bv = (np.asarray(v, dtype=np.float32) for v in (bq, bk, bv))
    # hidden/weights are pre-cast on the host to the projection matmul dtype
    # (same round-to-nearest the device casting DMA would apply)
    pj_np = mybir.dt.np(_mm_dts(MM_DTYPE)[0])
    hid_t = hid_t.astype(pj_np)
    in_maps = []
    for c in range(NCORES):
        rows = slice(c * CW, (c + 1) * CW)
        in_maps.append({
            "hidden_t": hid_t,
            "wqt": np.ascontiguousarray(Wq[rows, :].T).astype(pj_np),
            "wkt": np.ascontiguousarray(Wk[rows, :].T).astype(pj_np),
            "wvt": np.ascontiguousarray(Wv[rows, :].T).astype(pj_np),
            "bq": np.ascontiguousarray(bq[rows]),
            "bk": np.ascontiguousarray(bk[rows]),
            "bv": np.ascontiguousarray(bv[rows]),
            "mask": mask,
        })
    return in_maps


_NC_CACHE = {}


def _get_program():
    key = (B, S, MM_DTYPE)
    if key not in _NC_CACHE:
        _NC_CACHE[key] = build_program(B, S, MM_DTYPE)
    return _NC_CACHE[key]


def assemble_out(results, bv, b_sz=B, s_sz=S):
    """results[c]["out"] is [2, 65, n_tok] fp32 (num rows 0:64, den row 64,
    feature-major). Host finishes the softmax division, adds the V bias
    (sum_k (V+bv)*E = num + bv*den, so out = num/den + bv), transposes."""
    bv = np.asarray(bv, dtype=np.float32)
    out = np.empty((b_sz * s_sz, D), dtype=np.float32)
    for c in range(NCORES):
        o = results[c]["out"]
        for j in range(2):
            num = o[j, 0:HD, :]                      # [64, n_tok]
            den = o[j, HD, :]                        # [n_tok]
            col0 = c * CW + j * HD
            out[:, col0:col0 + HD] = (num / den).T + bv[col0:col0 + HD]
    return out.reshape(b_sz, s_sz, D)


def kernel(hidden_states, attention_mask, Wq, bq, Wk, bk, Wv, bv):
    from concourse.bass_utils import run_bass_kernel_spmd

    nc = _get_program()
    in_maps = make_in_maps(hidden_states, attention_mask, Wq, bq, Wk, bk, Wv,
                           bv)
    res = run_bass_kernel_spmd(nc, in_maps, list(range(NCORES)))
    return assemble_out(res.results, bv)



# revision 25
# speedup vs baseline: 1.0137x; 1.0103x over previous
"""BertSelfAttention fused kernel for Trainium2, 8 NeuronCores.

Sharding: tensor-parallel over heads. 16 heads / 8 cores = 2 heads per core.
Core c owns heads 2c, 2c+1 == output feature columns [128c, 128c+128).
Every core reads the full hidden_states (pre-transposed on host to [D, B*S])
plus its 128-column slice of Wq/Wk/Wv (pre-transposed to [D, 128]); it writes
a feature-major [2, 65, B*S] numerator/denominator slab, and the host
finishes the softmax division + transpose. No cross-core communication.

Per-core device program (B=4 batches, S=2048, D=1024, HD=64):
  stage 0: load weights/biases/mask constants; f = exp(mask) per key.
  per batch b (batch b+1's projections overlap batch b's attention; hid
  tiles prefetch on the gpsimd DMA queue):
    stage 1 (projection): QT,KT [128, 2048] (partition = head-elem dim,
      2 heads stacked), V [128 tok, 16 kblk, 130] where cols 0:64 head A,
      64 = denom col, 65:129 head B, 129 = denom col; V rows scaled by
      f=exp(mask) and the denom cols set to f, so the softmax denominator
      comes out of the PV matmul for free.
    stage 2 (attention), per 512-query group, software-pipelined at
    key-block granularity (PSUM: scores 2x2 banks, ctx 2, proj 2):
      for each 128-key block: S^T = K^T.T @ Q^T  (PSUM, keys on
        partitions; the two heads run as concurrent PE row-tiles)
        E = exp(S^T / 8)  (ACT, PSUM->SBUF fp16, one [128,1024]
        pass for both heads; 3 of 16 key-blocks instead compute E on the
        vector engine as a one-instruction fp16 Schraudolph to keep the
        saturated ACT off the critical path)
        ctx~ += [V|f].T @ E  (PSUM accumulate, [65, 512] per head;
                              row 64 = softmax denominator)
      epilogue: one DVE copy [65, 1024] PSUM->SBUF, two DMAs to the
        feature-major output slab.

Matmul operands are fp16 (1 PE cycle/column vs 2 for fp32r tf32 and 4
for fp32; fp16's 10 mantissa bits keep the output within ~5e-4 of the
fp32 reference before the Schraudolph blocks, ~8e-3 with them; all
tensors here fit fp16 range). PSUM accumulation is always fp32.
"""

import sys

sys.path.insert(0, "/opt/trn_rl_repo")

from contextlib import ExitStack

import numpy as np

import concourse.bass as bass
import concourse.mybir as mybir
import concourse.tile as tile
from concourse import bacc
from concourse.bass import ds
from concourse.masks import make_identity

B, S, D = 4, 2048, 1024
H, HD = 16, 64
NCORES = 8
CW = 128  # output columns per core (2 heads * 64)
P = 128

FP32 = mybir.dt.float32
FP32R = mybir.dt.float32r
BF16 = mybir.dt.bfloat16
FP16 = mybir.dt.float16

# matmul-operand dtype: "fp32" (exact, 4 cyc/col), "fp32r" (tf32, 2 cyc/col),
# "bf16"/"fp16" (1 cyc/col; fp16 carries 10 mantissa bits vs bf16's 8 and all
# tensors here fit fp16 range), "mixed" (projections fp32r, attention fp16)
MM_DTYPE = "fp16"

# key-blocks (of 16 per query group) whose exp runs on the vector engine as a
# one-instruction fp16 Schraudolph (bits = s*A + B, ~2% rel err on those
# blocks; full-pipeline numpy sim: 8.1e-3 final max rel err vs 2e-2 budget)
def _dve_exp_kbs(b, b_sz):
    return frozenset({2, 7, 12})
SCHRAUD_A = 1024.0 / float(np.log(2.0)) / 8.0   # fold the 1/8 score scale
SCHRAUD_B = 15360.0 - 44.0


def _mm_dts(mm_dtype):
    """-> (projection operand dtype, attention operand dtype)"""
    if mm_dtype == "mixed":
        return FP32R, FP16
    dt = {"fp32": FP32, "fp32r": FP32R, "bf16": BF16, "fp16": FP16}[mm_dtype]
    return dt, dt


def emit_kernel(ctx: ExitStack, tc: tile.TileContext, aps: dict, b_sz: int,
                s_sz: int, mm_dtype: str):
    nc = tc.nc
    n_tok = b_sz * s_sz
    TB = min(512, s_sz)           # projection token-block / query-group size
    n_tb = s_sz // TB             # token blocks per batch
    n_kb = s_sz // P              # key blocks per batch
    n_qg = s_sz // TB             # query groups per batch
    DCH = D // P                  # contraction chunks (8)
    n_bk = b_sz * n_kb            # total key blocks

    PJ, AT = _mm_dts(mm_dtype)
    hid_t, wqt, wkt, wvt, bq, bk, bv, mask, out = (
        aps["hidden_t"], aps["wqt"], aps["wkt"], aps["wvt"], aps["bq"],
        aps["bk"], aps["bv"], aps["mask"], aps["out"])

    const = ctx.enter_context(tc.tile_pool(name="const", bufs=1))
    hidp = ctx.enter_context(tc.tile_pool(name="hidp", bufs=4))
    qkv = ctx.enter_context(tc.tile_pool(name="qkv", bufs=6))
    epool = ctx.enter_context(tc.tile_pool(name="epool", bufs=6))
    csb = ctx.enter_context(tc.tile_pool(name="csb", bufs=3))
    vtmpp = ctx.enter_context(tc.tile_pool(name="vtmpp", bufs=2))
    psA = ctx.enter_context(tc.tile_pool(name="psA", bufs=2, space="PSUM"))
    psC = ctx.enter_context(tc.tile_pool(name="psC", bufs=1, space="PSUM"))
    psP = ctx.enter_context(tc.tile_pool(name="psP", bufs=2, space="PSUM"))

    # ---- stage 0: constants ----
    # Emission order tuned for startup: the first projection chain needs
    # wq + hid(0), so those DMAs go first on the queue; the identity (which
    # gates PE's first instruction, the mask transpose) is built on gpsimd
    # before the bv-broadcast DMA is queued there.
    wq_sb = const.tile([P, DCH, CW], PJ)
    nc.sync.dma_start(wq_sb, wqt.rearrange("(c p) m -> p c m", p=P))
    bq_sb = const.tile([P, 1], FP32)
    nc.sync.dma_start(bq_sb, bq.rearrange("(p o) -> p o", o=1))
    mask_bo = const.tile([n_bk, P], FP32)
    nc.sync.dma_start(mask_bo, mask.rearrange("b (o p) -> (b o) p", p=P))

    ident = const.tile([P, P], FP32)
    make_identity(nc, ident)

    # mask: [b_sz, s_sz] -> keys-on-partitions [128, b_sz * n_kb]
    mask_ps = psP.tile([P, n_bk], FP32, tag="proj", name="mask_ps")
    nc.tensor.matmul(mask_ps, mask_bo, ident[:n_bk, :n_bk], is_transpose=True)
    f_sb = const.tile([P, n_bk], FP32)
    nc.scalar.activation(f_sb, mask_ps, mybir.ActivationFunctionType.Exp)

    wk_sb = const.tile([P, DCH, CW], PJ)
    nc.sync.dma_start(wk_sb, wkt.rearrange("(c p) m -> p c m", p=P))
    wv_sb = const.tile([P, DCH, CW], PJ)
    nc.sync.dma_start(wv_sb, wvt.rearrange("(c p) m -> p c m", p=P))
    bk_sb = const.tile([P, 1], FP32)
    nc.sync.dma_start(bk_sb, bk.rearrange("(p o) -> p o", o=1))
    # bv broadcast to all partitions: [128, 128], every row = bv
    bvb = const.tile([P, CW], FP32)
    nc.gpsimd.dma_start(
        out=bvb,
        in_=bass.AP(tensor=bv.tensor, offset=bv.offset, ap=[[0, P], bv.ap[0]]),
    )

    for b in range(b_sz):
        # ---- stage 1: projections for batch b ----
        qt_b = qkv.tile([P, s_sz], AT, tag="qt", name="qt_b")
        kt_b = qkv.tile([P, s_sz], AT, tag="kt", name="kt_b")
        v_b = qkv.tile([P, n_kb, 130], AT, tag="v", name="v_b")

        for tb in range(n_tb):
            tok0 = b * s_sz + tb * TB
            hid_tile = hidp.tile([P, DCH, TB], PJ, tag="hid", name="hid_tile")
            # two half-depth DMAs so the first accumulation chunks can start
            # before the whole [1024, TB] slab has landed
            # hid tiles ride the (otherwise idle) gpsimd DMA queue so they
            # never sit behind weight/const/output DMAs on the sync queue
            hid_src = hid_t.rearrange("(c p) n -> p c n", p=P)[:, :, ds(tok0, TB)]
            nc.gpsimd.dma_start(hid_tile[:, 0:DCH // 2], hid_src[:, 0:DCH // 2])
            nc.gpsimd.dma_start(hid_tile[:, DCH // 2:DCH],
                                hid_src[:, DCH // 2:DCH])

            pq = psP.tile([P, TB], FP32, tag="proj", name="pq")
            for c in range(DCH):
                nc.tensor.matmul(pq, wq_sb[:, c, :],
                                 hid_tile[:, c, :],
                                 start=(c == 0), stop=(c == DCH - 1))
            nc.vector.tensor_scalar_add(qt_b[:, ds(tb * TB, TB)], pq, bq_sb)

            pk = psP.tile([P, TB], FP32, tag="proj", name="pk")
            for c in range(DCH):
                nc.tensor.matmul(pk, wk_sb[:, c, :],
                                 hid_tile[:, c, :],
                                 start=(c == 0), stop=(c == DCH - 1))
            nc.vector.tensor_scalar_add(kt_b[:, ds(tb * TB, TB)], pk, bk_sb)

            for s4 in range(TB // P):
                kbg = tb * (TB // P) + s4  # key block index within batch
                pv = psP.tile([P, CW], FP32, tag="proj", name="pv")
                for c in range(DCH):
                    nc.tensor.matmul(
                        pv, hid_tile[:, c, ds(s4 * P, P)],
                        wv_sb[:, c, :],
                        start=(c == 0), stop=(c == DCH - 1))
                vtmp = vtmpp.tile([P, CW], FP32, tag="vtmp", name="vtmp")
                nc.vector.tensor_add(vtmp, pv, bvb)
                fcol = f_sb[:, ds(b * n_kb + kbg, 1)]
                nc.vector.tensor_scalar_mul(v_b[:, kbg, 0:HD], vtmp[:, 0:HD],
                                            fcol)
                nc.vector.tensor_scalar_mul(v_b[:, kbg, 65:129],
                                            vtmp[:, HD:CW], fcol)
                nc.vector.tensor_copy(v_b[:, kbg, ds(HD, 1)], fcol)
                nc.vector.tensor_copy(v_b[:, kbg, ds(129, 1)], fcol)

        # ---- stage 2: attention for batch b ----
        for qg in range(n_qg):
            q0 = qg * TB
            ctx_ps = psC.tile([P, 2 * TB], FP32, tag="ctx", name="ctx_ps")

            def emit_scores(kb):
                st = psA.tile([P, 2 * TB], FP32, tag="st", name="st")
                nc.tensor.matmul(st[:, 0:TB],
                                 kt_b[0:HD, ds(kb * P, P)],
                                 qt_b[0:HD, ds(q0, TB)],
                                 start=True, stop=True)
                nc.tensor.matmul(st[:, ds(TB, TB)],
                                 kt_b[HD:P, ds(kb * P, P)],
                                 qt_b[HD:P, ds(q0, TB)],
                                 start=True, stop=True)
                return st

            # software-pipelined with a single sem gate per kb on the PE
            # queue: exp(kb)-done unblocks BOTH pv(kb) and scores(kb+2) (the
            # latter reuses the st buffer exp(kb) just drained), so the PE
            # pays one resume-latency tail per kb instead of two
            sts = [emit_scores(0), emit_scores(1)]
            for kb in range(n_kb):
                e_t = epool.tile([P, 2 * TB], AT, tag="e", name="e_t")
                if kb in _dve_exp_kbs(b, b_sz) and AT == FP16:
                    # fp16 Schraudolph on the vector engine: the fp16 bit
                    # pattern of ~exp(s/8) is the affine int16(s*A + B);
                    # offloads 3/16 of the exp stream from the saturated ACT
                    nc.vector.tensor_scalar(
                        e_t.bitcast(mybir.dt.int16), sts[kb],
                        SCHRAUD_A, SCHRAUD_B,
                        mybir.AluOpType.mult, mybir.AluOpType.add)
                else:
                    nc.scalar.activation(e_t, sts[kb],
                                         mybir.ActivationFunctionType.Exp,
                                         scale=1.0 / 8.0)
                nc.tensor.matmul(ctx_ps[0:65, 0:TB],
                                 v_b[:, kb, 0:65],
                                 e_t[:, 0:TB],
                                 start=(kb == 0), stop=(kb == n_kb - 1))
                nc.tensor.matmul(ctx_ps[0:65, ds(TB, TB)],
                                 v_b[:, kb, ds(65, 65)],
                                 e_t[:, ds(TB, TB)],
                                 start=(kb == 0), stop=(kb == n_kb - 1))
                if kb + 2 < n_kb:
                    sts.append(emit_scores(kb + 2))

            # evacuate the [65, 2*TB] numerator/denominator block to SBUF and
            # DMA it out feature-major; the host does num/den + transpose
            ctx_sb = csb.tile([65, 2 * TB], FP32, tag="csb", name="ctx_sb")
            nc.vector.tensor_copy(ctx_sb, ctx_ps[0:65, :])
            tok0 = b * s_sz + q0
            for j in range(2):
                nc.sync.dma_start(out[j, :, ds(tok0, TB)],
                                  ctx_sb[:, ds(j * TB, TB)])


def build_program(b_sz=B, s_sz=S, mm_dtype=None):
    if mm_dtype is None:
        mm_dtype = MM_DTYPE
    PJ, _ = _mm_dts(mm_dtype)
    nc = bacc.Bacc("TRN2", target_bir_lowering=False, debug=False)
    n_tok = b_sz * s_sz
    aps = {
        "hidden_t": nc.dram_tensor("hidden_t", [D, n_tok], PJ,
                                   kind="ExternalInput").ap(),
        "wqt": nc.dram_tensor("wqt", [D, CW], PJ, kind="ExternalInput").ap(),
        "wkt": nc.dram_tensor("wkt", [D, CW], PJ, kind="ExternalInput").ap(),
        "wvt": nc.dram_tensor("wvt", [D, CW], PJ, kind="ExternalInput").ap(),
        "bq": nc.dram_tensor("bq", [CW], FP32, kind="ExternalInput").ap(),
        "bk": nc.dram_tensor("bk", [CW], FP32, kind="ExternalInput").ap(),
        "bv": nc.dram_tensor("bv", [CW], FP32, kind="ExternalInput").ap(),
        "mask": nc.dram_tensor("mask", [b_sz, s_sz], FP32,
                               kind="ExternalInput").ap(),
        # feature-major numerator+denominator: out[j, 0:64, tok] = head-j
        # context numerator, out[j, 64, tok] = softmax denominator
        "out": nc.dram_tensor("out", [2, 65, n_tok], FP32,
                              kind="ExternalOutput").ap(),
    }
    with tile.TileContext(nc) as tc:
        with ExitStack() as ctx:
            emit_kernel(ctx, tc, aps, b_sz, s_sz, mm_dtype)
    nc.compile()
    return nc


def make_in_maps(hidden_states, attention_mask, Wq, bq, Wk, bk, Wv, bv,
                 b_sz=B, s_sz=S):
    hidden_states = np.asarray(hidden_states, dtype=np.float32)
    x = hidden_states.reshape(b_sz * s_sz, D)
    hid_t = np.ascontiguousarray(x.T)
    mask = np.ascontiguousarray(
        np.broadcast_to(
            np.asarray(attention_mask, dtype=np.float32).reshape(b_sz, 1, 1,
                                                                 s_sz),
            (b_sz, 1, 1, s_sz)).reshape(b_sz, s_sz))
    Wq, Wk, Wv = (np.asarray(w, dtype=np.float32) for w in (Wq, Wk, Wv))
    bq, bk, bv = (np.asarray(v, dtype=np.float32) for v in (bq, bk, bv))
    # hidden/weights are pre-cast on the host to the projection matmul dtype
    # (same round-to-nearest the device casting DMA would apply)
    pj_np = mybir.dt.np(_mm_dts(MM_DTYPE)[0])
    hid_t = hid_t.astype(pj_np)
    in_maps = []
    for c in range(NCORES):
        rows = slice(c * CW, (c + 1) * CW)
        in_maps.append({
            "hidden_t": hid_t,
            "wqt": np.ascontiguousarray(Wq[rows, :].T).astype(pj_np),
            "wkt": np.ascontiguousarray(Wk[rows, :].T).astype(pj_np),
            "wvt": np.ascontiguousarray(Wv[rows, :].T).astype(pj_np),
            "bq": np.ascontiguousarray(bq[rows]),
            "bk": np.ascontiguousarray(bk[rows]),
            "bv": np.ascontiguousarray(bv[rows]),
            "mask": mask,
        })
    return in_maps


_NC_CACHE = {}


def _get_program():
    key = (B, S, MM_DTYPE)
    if key not in _NC_CACHE:
        _NC_CACHE[key] = build_program(B, S, MM_DTYPE)
    return _NC_CACHE[key]


def assemble_out(results, b_sz=B, s_sz=S):
    """results[c]["out"] is [2, 65, n_tok] fp32 (num rows 0:64, den row 64,
    feature-major). Host finishes the softmax division + transpose."""
    out = np.empty((b_sz * s_sz, D), dtype=np.float32)
    for c in range(NCORES):
        o = results[c]["out"]
        for j in range(2):
            num = o[j, 0:HD, :]                      # [64, n_tok]
            den = o[j, HD, :]                        # [n_tok]
            col0 = c * CW + j * HD
            out[:, col0:col0 + HD] = (num / den).T
    return out.reshape(b_sz, s_sz, D)


def kernel(hidden_states, attention_mask, Wq, bq, Wk, bk, Wv, bv):
    from concourse.bass_utils import run_bass_kernel_spmd

    nc = _get_program()
    in_maps = make_in_maps(hidden_states, attention_mask, Wq, bq, Wk, bk, Wv,
                           bv)
    res = run_bass_kernel_spmd(nc, in_maps, list(range(NCORES)))
    return assemble_out(res.results)

